# revision 23
# baseline (speedup 1.0000x reference)
"""EvidenceQualityLayer Trainium2 kernel — 8-core head+batch parallel.

Sharding: each of the 8 cores owns 2 heads (128 features) of the Q/K/V
projections (column-parallel) and computes its heads' full SxS evidence
tensor for both batch elements in a flash-attention-style streaming loop.
The attention output shards (feature-major, [128, S] per batch) are
AllGather'ed across cores, after which each core computes a 128-column
slice of the output projection (column-parallel out_proj).  No all-reduce
is needed anywhere.

Math identity used (no softmax-max trick needed, raw exp):
  evidence = exp(s)*scale + (1+bias)
  denom    = scale*rowsum(exp(s)) + S*(1+bias)
  attn     = (scale*(E @ V') + (1+bias)*colsum(V')) / denom
  unc      = S / denom
where V' = V + v_b (probs rows sum to 1 exactly, so folding v_b into V
is exact).  rowsum(E) comes for free from a ones-column appended to V.

All matmuls are fp16 operands with fp32 PSUM accumulation (~1e-3 rel
err, 4x faster than fp32 on the PE).  Both P2 matmul shapes are padded
to use the full 128x128 PE array (zero rows in K^T, zero cols in the V
stationary): half-array matmuls never trip the PE clock-gate's activity
monitor and the whole phase runs at 1.2 GHz instead of 2.4.
"""

import os

import numpy as np

B = 2
S = 2048
D = 1024
H = 16
Hd = 64
NCORES = 8
P = 128
HPD = H // NCORES      # heads per device (2)
FPD = HPD * Hd         # features per device (128)
QT = 4                 # q tiles of 512 per sequence
KT = S // P            # k tiles of 128 (16)
ET = D // P            # embed contraction chunks (8)
VW = 192               # V stationary layout: V0(64) | ones(64) | V1(64)

_CACHE: dict = {}


def _build_nc():
    import concourse.bacc as bacc
    import concourse.mybir as mybir
    import concourse.tile as tile

    fp16 = mybir.dt.float16
    fp32 = mybir.dt.float32
    Alu = mybir.AluOpType
    ActF = mybir.ActivationFunctionType

    nc = bacc.Bacc(num_devices=NCORES, name="evq")

    # ---- I/O ----
    xT = nc.dram_tensor("xT", [D, B * S], fp16, kind="ExternalInput")
    wq = nc.dram_tensor("wq", [P, ET * FPD], fp16, kind="ExternalInput")
    wk = nc.dram_tensor("wk", [P, ET * FPD], fp16, kind="ExternalInput")
    wv = nc.dram_tensor("wv", [P, ET * FPD], fp16, kind="ExternalInput")
    wo = nc.dram_tensor("wo", [P, ET * FPD], fp16, kind="ExternalInput")
    bq = nc.dram_tensor("bq", [FPD, 1], fp32, kind="ExternalInput")
    bk = nc.dram_tensor("bk", [FPD, 1], fp32, kind="ExternalInput")
    bv = nc.dram_tensor("bv", [FPD, 1], fp32, kind="ExternalInput")
    bo = nc.dram_tensor("bo", [FPD, 1], fp32, kind="ExternalInput")
    # sc columns: 0 = evidence_scale, 1 = (1+evidence_bias), 2 = S*(1+bias)
    sc = nc.dram_tensor("sc", [P, 3], fp32, kind="ExternalInput")
    ident = nc.dram_tensor("ident", [P, P], fp16, kind="ExternalInput")

    outT = nc.dram_tensor("outT", [FPD, B * S], fp32, kind="ExternalOutput")
    unc = nc.dram_tensor("unc", [B * HPD, S], fp32, kind="ExternalOutput")

    with tile.TileContext(nc) as tc:
        with (
            tc.tile_pool(name="consts", bufs=1) as cpool,
            tc.tile_pool(name="proj", bufs=1) as ppool,
            tc.tile_pool(name="dram", bufs=1, space="DRAM") as dpool,
        ):
            # ---- constants into SBUF ----
            wq_sb = cpool.tile([P, ET, FPD], fp16, name="wq_sb")
            wk_sb = cpool.tile([P, ET, FPD], fp16, name="wk_sb")
            wv_sb = cpool.tile([P, ET, FPD], fp16, name="wv_sb")
            wo_sb = cpool.tile([P, ET, FPD], fp16, name="wo_sb")
            for w_sb, w_dr in ((wq_sb, wq), (wk_sb, wk), (wv_sb, wv), (wo_sb, wo)):
                nc.gpsimd.dma_start(
                    w_sb[:], w_dr[:].rearrange("p (o m) -> p o m", o=ET)
                )
            bq_sb = cpool.tile([FPD, 1], fp32, name="bq_sb")
            bk_sb = cpool.tile([FPD, 1], fp32, name="bk_sb")
            bv_sb = cpool.tile([FPD, 1], fp32, name="bv_sb")
            bo_sb = cpool.tile([FPD, 1], fp32, name="bo_sb")
            for b_sb, b_dr in ((bq_sb, bq), (bk_sb, bk), (bv_sb, bv), (bo_sb, bo)):
                nc.gpsimd.dma_start(b_sb[:], b_dr[:])
            sc_sb = cpool.tile([P, 3], fp32, name="sc_sb")
            nc.gpsimd.dma_start(sc_sb[:], sc[:])
            id_sb = cpool.tile([P, P], fp16, name="id_sb")
            nc.gpsimd.dma_start(id_sb[:], ident[:])
            ones_sb = cpool.tile([P, 1], fp16, name="ones_sb")
            nc.vector.memset(ones_sb[:], 1.0)
            srow_sb = cpool.tile([1, S], fp32, name="srow_sb")
            nc.vector.memset(srow_sb[:], float(S))

            # ---- persistent per-phase tiles ----
            qT_sb = ppool.tile([P, B, S], fp16, name="qT_sb")
            # K^T packed like Q^T: head h on partition rows [64h:64h+64]
            kT_sb = ppool.tile([P, B, S], fp16, name="kT_sb")
            # V stationary per (b, kt): [V0(64) | ones(64) | V1(64)].
            # Head slice h -> cols [64h : 64h+128] = [V_h | ones] (h0) or
            # [ones | V_h] (h1), so the EV matmul's 128-row PSUM output
            # carries EV on one 64-row half and 64 replicated rowsum(E)
            # rows on the other half -> full PE array AND no partition
            # broadcast needed for the denominator.
            v_sb = ppool.tile([P, B, KT, VW], fp16, name="v_sb")
            for b in range(B):
                for kt in range(KT):
                    nc.vector.memset(v_sb[:, b, kt, Hd:2 * Hd], 1.0)
            # (1+bias)*colsum(V') per (b, head), aligned to the head's
            # EV rows (h0 -> partitions 0:64, h1 -> partitions 64:128)
            cc_sb = ppool.tile([P, B * HPD], fp32, name="cc_sb")
            # attention output shards, feature-major [64, S] per (b, h)
            at_sb = [
                [
                    ppool.tile([Hd, S], fp16, name=f"at_{b}_{h}")
                    for h in range(HPD)
                ]
                for b in range(B)
            ]
            unc_sb = [
                ppool.tile([1, S], fp32, name=f"unc_sb_{i}")
                for i in range(B * HPD)
            ]

            # ================= P1: projections =================
            # o-outer loop: one stationary weight chunk feeds all 8
            # (b, qt) accumulators -> dense full-array MM stream.
            # xpool (x^T + V^T temps) is scoped to P1 so its 72 KB/partition
            # is reused by the P2/P3 working pool.
            xpool = tc.alloc_tile_pool(name="xp", bufs=1)
            xT_sb = xpool.tile([P, ET, B * S], fp16, name="xT_sb")
            for o in range(ET):
                nc.sync.dma_start(xT_sb[:, o, :], xT[o * P:(o + 1) * P, :])

            with tc.tile_pool(name="psA1", bufs=1, space="PSUM") as psA1:
                vT_tmp = [
                    xpool.tile([P, S], fp16, name=f"vT_{b}") for b in range(B)
                ]
                for proj_i, (w_sb, b_sb) in enumerate(
                    ((wq_sb, bq_sb), (wk_sb, bk_sb), (wv_sb, bv_sb))
                ):
                    pp = [
                        psA1.tile(
                            [P, 512], fp32, tag="proj", bufs=8,
                            name=f"pp_{proj_i}_{i}",
                        )
                        for i in range(B * QT)
                    ]
                    for o in range(ET):
                        for b in range(B):
                            for qt in range(QT):
                                col = b * S + qt * 512
                                nc.tensor.matmul(
                                    pp[b * QT + qt][:],
                                    w_sb[:, o, :],
                                    xT_sb[:, o, col:col + 512],
                                    start=(o == 0),
                                    stop=(o == ET - 1),
                                )
                    for b in range(B):
                        for qt in range(QT):
                            p_t = pp[b * QT + qt]
                            q0 = qt * 512
                            if proj_i == 0:
                                nc.vector.tensor_scalar_add(
                                    qT_sb[:, b, q0:q0 + 512], p_t[:], b_sb[:]
                                )
                            elif proj_i == 1:
                                nc.vector.tensor_scalar_add(
                                    kT_sb[:, b, q0:q0 + 512], p_t[:], b_sb[:]
                                )
                            else:
                                nc.vector.tensor_scalar_add(
                                    vT_tmp[b][:, q0:q0 + 512], p_t[:], b_sb[:]
                                )

            with tc.tile_pool(name="psA2", bufs=1, space="PSUM") as psA2:
                # transpose V^T -> V (split heads into the padded layout);
                # colsum(V') = free-axis reduce of V^T on the DVE (the d axis
                # is already partition-major there) — no PE involvement.
                for b in range(B):
                    csum = ppool.tile([P, 1], fp32, name=f"csum_{b}")
                    nc.vector.tensor_reduce(
                        csum[:], vT_tmp[b][:], mybir.AxisListType.X, Alu.add
                    )
                    nc.vector.tensor_scalar_mul(
                        cc_sb[:, b * HPD:b * HPD + 1], csum[:], sc_sb[:, 1:2]
                    )
                    nc.vector.tensor_copy(
                        out=cc_sb[:, b * HPD + 1:b * HPD + 2],
                        in_=cc_sb[:, b * HPD:b * HPD + 1],
                    )
                    for kt in range(KT):
                        pt = psA2.tile(
                            [P, P], fp16, tag="tr", bufs=3, name=f"pt_{b}_{kt}"
                        )
                        nc.tensor.transpose(
                            pt[:], vT_tmp[b][:, kt * P:(kt + 1) * P], id_sb[:]
                        )
                        nc.vector.tensor_copy(
                            out=v_sb[:, b, kt, 0:Hd], in_=pt[:, 0:Hd]
                        )
                        nc.vector.tensor_copy(
                            out=v_sb[:, b, kt, 2 * Hd:3 * Hd], in_=pt[:, Hd:P]
                        )

            xpool.release()

            # ================= P2: evidence attention =================
            wpool = tc.alloc_tile_pool(name="work", bufs=1)
            # per (b, S-half) AllGather: fires as soon as the first two
            # q-tiles of both heads are done -> overlaps remaining compute
            ag_in = [
                [
                    dpool.tile([FPD, S // 2], fp16, name=f"ag_in_{b}_{sh}")
                    for sh in range(2)
                ]
                for b in range(B)
            ]
            ag_out = [
                [
                    dpool.tile(
                        [NCORES * FPD, S // 2], fp16, addr_space="Shared",
                        name=f"ag_out_{b}_{sh}",
                    )
                    for sh in range(2)
                ]
                for b in range(B)
            ]

            with tc.tile_pool(name="psB", bufs=1, space="PSUM") as psB:
                def _emit_p3(b):
                    # st-outer: one [128,512] accumulator at a time from the
                    # shared "ev" tag, so these matmuls interleave into the
                    # other batch's P2 stream as soon as the AllGather lands.
                    for st in range(QT):
                        po = psB.tile(
                            [P, 512], fp32, tag="po", bufs=1,
                            name=f"po_{b}_{st}",
                        )
                        sh = st // 2
                        for fc in range(ET):
                            agc = wpool.tile(
                                [P, 512], fp16, tag="agc", bufs=4,
                                name=f"agc_{b}_{st}_{fc}",
                            )
                            nc.sync.dma_start(
                                agc[:],
                                ag_out[b][sh][fc * P:(fc + 1) * P,
                                              (st % 2) * 512:
                                              (st % 2) * 512 + 512],
                            )
                            nc.tensor.matmul(
                                po[:],
                                wo_sb[:, fc, :],
                                agc[:],
                                start=(fc == 0),
                                stop=(fc == ET - 1),
                            )
                        ot = wpool.tile(
                            [P, 512], fp32, tag="ot", bufs=3,
                            name=f"ot_{b}_{st}",
                        )
                        nc.vector.tensor_scalar_add(ot[:], po[:], bo_sb[:])
                        nc.sync.dma_start(
                            outT[:, b * S + st * 512:b * S + (st + 1) * 512],
                            ot[:],
                        )

                for b in range(B):
                    for qt in range(QT):
                        q0 = qt * 512
                        # one [128,1024] scores tile holds BOTH heads for
                        # this q-tile; the two 64-contraction matmuls run
                        # concurrently on disjoint PE row groups.
                        ev = [
                            psB.tile(
                                [P, 512], fp32, tag="ev", bufs=3,
                                name=f"ev_{b}_{qt}_{h}",
                            )
                            for h in range(HPD)
                        ]
                        for kt in range(KT):
                            ss = psB.tile(
                                [P, 1024], fp32, tag="sc", bufs=2,
                                name=f"ss_{b}_{qt}_{kt}",
                            )
                            nc.tensor.matmul(
                                ss[:, 0:512],
                                kT_sb[0:Hd, b, kt * P:(kt + 1) * P],
                                qT_sb[0:Hd, b, q0:q0 + 512],
                                start=True,
                                stop=True,
                                tile_position=(0, 0),
                            )
                            nc.tensor.matmul(
                                ss[:, 512:1024],
                                kT_sb[Hd:P, b, kt * P:(kt + 1) * P],
                                qT_sb[Hd:P, b, q0:q0 + 512],
                                start=True,
                                stop=True,
                                tile_position=(Hd, 0),
                            )
                            et = wpool.tile(
                                [P, 1024], fp16, tag="et", bufs=4,
                                name=f"et_{b}_{qt}_{kt}",
                            )
                            nc.scalar.activation(
                                et[:], ss[:], ActF.Exp, scale=0.125
                            )
                            for h in range(HPD):
                                nc.tensor.matmul(
                                    ev[h][:],
                                    v_sb[:, b, kt, h * Hd:h * Hd + P],
                                    et[:, h * 512:(h + 1) * 512],
                                    start=(kt == 0),
                                    stop=(kt == KT - 1),
                                )
                        # ---- epilogue for this (b, qt), both heads ----
                        for h in range(HPD):
                            col = b * HPD + h
                            e_lo = h * Hd          # EV rows base
                            d_lo = Hd - h * Hd     # rowsum rows base
                            den = wpool.tile(
                                [Hd, 512], fp32, tag="den", bufs=2,
                                name=f"den_{b}_{qt}_{h}",
                            )
                            tmp = wpool.tile(
                                [Hd, 512], fp32, tag="tmp", bufs=2,
                                name=f"tmp_{b}_{qt}_{h}",
                            )
                            nc.vector.tensor_scalar(
                                den[:],
                                ev[h][d_lo:d_lo + Hd, :],
                                sc_sb[d_lo:d_lo + Hd, 0:1],
                                sc_sb[d_lo:d_lo + Hd, 2:3],
                                Alu.mult,
                                Alu.add,
                            )
                            nc.vector.tensor_scalar(
                                tmp[:],
                                ev[h][e_lo:e_lo + Hd, :],
                                sc_sb[e_lo:e_lo + Hd, 0:1],
                                cc_sb[e_lo:e_lo + Hd, col:col + 1],
                                Alu.mult,
                                Alu.add,
                            )
                            rb = wpool.tile(
                                [Hd, 512], fp32, tag="rb", bufs=2,
                                name=f"rb_{b}_{qt}_{h}",
                            )
                            rs = wpool.tile(
                                [Hd, 512], fp32, tag="rs", bufs=2,
                                name=f"rs_{b}_{qt}_{h}",
                            )
                            nc.vector.reciprocal_approx_accurate(
                                rb[:], den[:], rs[:]
                            )
                            nc.vector.tensor_tensor(
                                at_sb[b][h][:, q0:q0 + 512],
                                tmp[:],
                                rb[:],
                                Alu.mult,
                            )
                            nc.vector.tensor_scalar_mul(
                                unc_sb[col][:, q0:q0 + 512],
                                rb[0:1, :],
                                float(S),
                            )
                        if qt % 2 == 1:
                            sh = qt // 2
                            s0 = sh * (S // 2)
                            for h in range(HPD):
                                nc.sync.dma_start(
                                    ag_in[b][sh][h * Hd:(h + 1) * Hd, :],
                                    at_sb[b][h][:, s0:s0 + S // 2],
                                )
                            nc.gpsimd.collective_compute(
                                "AllGather",
                                mybir.AluOpType.bypass,
                                replica_groups=[list(range(NCORES))],
                                ins=[ag_in[b][sh].opt()],
                                outs=[ag_out[b][sh].opt()],
                            )

                for b in range(B):
                    _emit_p3(b)
            for i in range(B * HPD):
                nc.sync.dma_start(unc[i:i + 1, :], unc_sb[i][:])
            wpool.release()

    nc.finalize()
    return nc


def _get_nc():
    if "nc" not in _CACHE:
        _CACHE["nc"] = _build_nc()
    return _CACHE["nc"]


def kernel(
    x, q_w, q_b, k_w, k_b, v_w, v_b, out_w, out_b,
    evidence_scale, evidence_bias,
):
    from concourse.bass_utils import run_bass_kernel_spmd

    x = np.asarray(x, dtype=np.float32)
    q_w = np.asarray(q_w, dtype=np.float32)
    k_w = np.asarray(k_w, dtype=np.float32)
    v_w = np.asarray(v_w, dtype=np.float32)
    out_w = np.asarray(out_w, dtype=np.float32)
    q_b = np.asarray(q_b, dtype=np.float32)
    k_b = np.asarray(k_b, dtype=np.float32)
    v_b = np.asarray(v_b, dtype=np.float32)
    out_b = np.asarray(out_b, dtype=np.float32)
    scale = float(np.asarray(evidence_scale).reshape(-1)[0])
    ebias = float(np.asarray(evidence_bias).reshape(-1)[0])

    xTh = np.ascontiguousarray(
        x.reshape(B * S, D).T
    ).astype(np.float16)                                   # [D, B*S]
    ident = np.eye(P, dtype=np.float16)
    sc_host = np.empty((P, 3), dtype=np.float32)
    sc_host[:, 0] = scale
    sc_host[:, 1] = 1.0 + ebias
    sc_host[:, 2] = float(S) * (1.0 + ebias)

    def _wlayout(a):
        # a: [FPD, D] weight shard; device wants w_sb[p, o, m] = a[m, o*P+p]
        return np.ascontiguousarray(
            a.reshape(FPD, ET, P).transpose(2, 1, 0).reshape(P, ET * FPD)
        ).astype(np.float16)

    in_maps = []
    for d in range(NCORES):
        f0 = d * FPD
        sl = slice(f0, f0 + FPD)
        in_maps.append({
            "xT": xTh,
            "wq": _wlayout(q_w[sl, :]),
            "wk": _wlayout(k_w[sl, :]),
            "wv": _wlayout(v_w[sl, :]),
            "wo": _wlayout(out_w[sl, :]),
            "bq": np.ascontiguousarray(q_b[sl].reshape(FPD, 1)),
            "bk": np.ascontiguousarray(k_b[sl].reshape(FPD, 1)),
            "bv": np.ascontiguousarray(v_b[sl].reshape(FPD, 1)),
            "bo": np.ascontiguousarray(out_b[sl].reshape(FPD, 1)),
            "sc": sc_host,
            "ident": ident,
        })

    nc = _get_nc()
    trace = bool(int(os.environ.get("EVQ_TRACE", "0")))
    res = run_bass_kernel_spmd(
        nc, in_maps, core_ids=list(range(NCORES)), trace=trace,
    )
    if trace and res.exec_time_ns is not None:
        print(f"HW exec time: {res.exec_time_ns} ns")
        if res.instructions_and_trace is not None:
            print(f"trace: {res.instructions_and_trace[1]}")
        _CACHE["last_result"] = res

    out = np.empty((B, S, D), dtype=np.float32)
    uncertainty = np.empty((B, H, S), dtype=np.float32)
    for d in range(NCORES):
        r = res.results[d]
        oT = r["outT"].reshape(FPD, B, S)          # [128, B, S]
        for b in range(B):
            out[b, :, d * FPD:(d + 1) * FPD] = oT[:, b, :].T
        u = r["unc"]                               # [B*HPD, S]
        for b in range(B):
            for h in range(HPD):
                uncertainty[b, d * HPD + h, :] = u[b * HPD + h, :]
    return out, uncertainty


# revision 24
# speedup vs baseline: 1.0312x; 1.0312x over previous
"""EvidenceQualityLayer Trainium2 kernel — 8-core head+batch parallel.

Sharding: each of the 8 cores owns 2 heads (128 features) of the Q/K/V
projections (column-parallel) and computes its heads' full SxS evidence
tensor for both batch elements in a flash-attention-style streaming loop.
The attention output shards (feature-major, [128, S] per batch) are
AllGather'ed across cores, after which each core computes a 128-column
slice of the output projection (column-parallel out_proj).  No all-reduce
is needed anywhere.

Math identity used (no softmax-max trick needed, raw exp):
  evidence = exp(s)*scale + (1+bias)
  denom    = scale*rowsum(exp(s)) + S*(1+bias)
  attn     = (scale*(E @ V') + (1+bias)*colsum(V')) / denom
  unc      = S / denom
where V' = V + v_b (probs rows sum to 1 exactly, so folding v_b into V
is exact).  rowsum(E) comes for free from a ones-column appended to V.

All matmuls are fp16 operands with fp32 PSUM accumulation (~1e-3 rel
err, 4x faster than fp32 on the PE).  Both P2 matmul shapes are padded
to use the full 128x128 PE array (zero rows in K^T, zero cols in the V
stationary): half-array matmuls never trip the PE clock-gate's activity
monitor and the whole phase runs at 1.2 GHz instead of 2.4.
"""

import os

import numpy as np

B = 2
S = 2048
D = 1024
H = 16
Hd = 64
NCORES = 8
P = 128
HPD = H // NCORES      # heads per device (2)
FPD = HPD * Hd         # features per device (128)
QT = 4                 # q tiles of 512 per sequence
KT = S // P            # k tiles of 128 (16)
ET = D // P            # embed contraction chunks (8)
VW = 192               # V stationary layout: V0(64) | ones(64) | V1(64)

_CACHE: dict = {}


def _build_nc():
    import concourse.bacc as bacc
    import concourse.mybir as mybir
    import concourse.tile as tile

    fp16 = mybir.dt.float16
    fp32 = mybir.dt.float32
    Alu = mybir.AluOpType
    ActF = mybir.ActivationFunctionType

    nc = bacc.Bacc(num_devices=NCORES, name="evq")

    # ---- I/O ----
    xT = nc.dram_tensor("xT", [D, B * S], fp16, kind="ExternalInput")
    wq = nc.dram_tensor("wq", [P, ET * FPD], fp16, kind="ExternalInput")
    wk = nc.dram_tensor("wk", [P, ET * FPD], fp16, kind="ExternalInput")
    wv = nc.dram_tensor("wv", [P, ET * FPD], fp16, kind="ExternalInput")
    wo = nc.dram_tensor("wo", [P, ET * FPD], fp16, kind="ExternalInput")
    bq = nc.dram_tensor("bq", [FPD, 1], fp32, kind="ExternalInput")
    bk = nc.dram_tensor("bk", [FPD, 1], fp32, kind="ExternalInput")
    bv = nc.dram_tensor("bv", [FPD, 1], fp32, kind="ExternalInput")
    bo = nc.dram_tensor("bo", [FPD, 1], fp32, kind="ExternalInput")
    # sc columns: 0 = evidence_scale, 1 = (1+evidence_bias), 2 = S*(1+bias)
    sc = nc.dram_tensor("sc", [P, 3], fp32, kind="ExternalInput")
    ident = nc.dram_tensor("ident", [P, P], fp16, kind="ExternalInput")

    outT = nc.dram_tensor("outT", [FPD, B * S], fp32, kind="ExternalOutput")
    unc = nc.dram_tensor("unc", [B * HPD, S], fp32, kind="ExternalOutput")

    with tile.TileContext(nc) as tc:
        with (
            tc.tile_pool(name="consts", bufs=1) as cpool,
            tc.tile_pool(name="proj", bufs=1) as ppool,
            tc.tile_pool(name="dram", bufs=1, space="DRAM") as dpool,
        ):
            # ---- constants into SBUF ----
            wq_sb = cpool.tile([P, ET, FPD], fp16, name="wq_sb")
            wk_sb = cpool.tile([P, ET, FPD], fp16, name="wk_sb")
            wv_sb = cpool.tile([P, ET, FPD], fp16, name="wv_sb")
            wo_sb = cpool.tile([P, ET, FPD], fp16, name="wo_sb")
            for w_sb, w_dr in ((wq_sb, wq), (wk_sb, wk), (wv_sb, wv), (wo_sb, wo)):
                nc.gpsimd.dma_start(
                    w_sb[:], w_dr[:].rearrange("p (o m) -> p o m", o=ET)
                )
            bq_sb = cpool.tile([FPD, 1], fp32, name="bq_sb")
            bk_sb = cpool.tile([FPD, 1], fp32, name="bk_sb")
            bv_sb = cpool.tile([FPD, 1], fp32, name="bv_sb")
            bo_sb = cpool.tile([FPD, 1], fp32, name="bo_sb")
            for b_sb, b_dr in ((bq_sb, bq), (bk_sb, bk), (bv_sb, bv), (bo_sb, bo)):
                nc.gpsimd.dma_start(b_sb[:], b_dr[:])
            sc_sb = cpool.tile([P, 3], fp32, name="sc_sb")
            nc.gpsimd.dma_start(sc_sb[:], sc[:])
            id_sb = cpool.tile([P, P], fp16, name="id_sb")
            nc.gpsimd.dma_start(id_sb[:], ident[:])
            ones_sb = cpool.tile([P, 1], fp16, name="ones_sb")
            nc.vector.memset(ones_sb[:], 1.0)
            srow_sb = cpool.tile([1, S], fp32, name="srow_sb")
            nc.vector.memset(srow_sb[:], float(S))

            # ---- persistent per-phase tiles ----
            qT_sb = ppool.tile([P, B, S], fp16, name="qT_sb")
            # K^T packed like Q^T: head h on partition rows [64h:64h+64]
            kT_sb = ppool.tile([P, B, S], fp16, name="kT_sb")
            # V stationary per (b, kt): [V0(64) | ones(64) | V1(64)].
            # Head slice h -> cols [64h : 64h+128] = [V_h | ones] (h0) or
            # [ones | V_h] (h1), so the EV matmul's 128-row PSUM output
            # carries EV on one 64-row half and 64 replicated rowsum(E)
            # rows on the other half -> full PE array AND no partition
            # broadcast needed for the denominator.
            v_sb = ppool.tile([P, B, KT, VW], fp16, name="v_sb")
            for b in range(B):
                for kt in range(KT):
                    nc.vector.memset(v_sb[:, b, kt, Hd:2 * Hd], 1.0)
            # (1+bias)*colsum(V') per (b, head), aligned to the head's
            # EV rows (h0 -> partitions 0:64, h1 -> partitions 64:128)
            cc_sb = ppool.tile([P, B * HPD], fp32, name="cc_sb")
            # attention output shards, feature-major [64, S] per (b, h)
            at_sb = [
                [
                    ppool.tile([Hd, S], fp16, name=f"at_{b}_{h}")
                    for h in range(HPD)
                ]
                for b in range(B)
            ]
            unc_sb = [
                ppool.tile([1, S], fp32, name=f"unc_sb_{i}")
                for i in range(B * HPD)
            ]

            # ================= P1: projections =================
            # o-outer loop: one stationary weight chunk feeds all 8
            # (b, qt) accumulators -> dense full-array MM stream.
            # xpool (x^T + V^T temps) is scoped to P1 so its 72 KB/partition
            # is reused by the P2/P3 working pool.
            xpool = tc.alloc_tile_pool(name="xp", bufs=1)
            xT_sb = xpool.tile([P, ET, B * S], fp16, name="xT_sb")
            for o in range(ET):
                nc.sync.dma_start(xT_sb[:, o, :], xT[o * P:(o + 1) * P, :])

            with tc.tile_pool(name="psA1", bufs=1, space="PSUM") as psA1:
                vT_tmp = [
                    xpool.tile([P, S], fp16, name=f"vT_{b}") for b in range(B)
                ]
                for proj_i, (w_sb, b_sb) in enumerate(
                    ((wq_sb, bq_sb), (wk_sb, bk_sb), (wv_sb, bv_sb))
                ):
                    pp = [
                        psA1.tile(
                            [P, 512], fp32, tag="proj", bufs=8,
                            name=f"pp_{proj_i}_{i}",
                        )
                        for i in range(B * QT)
                    ]
                    for o in range(ET):
                        for b in range(B):
                            for qt in range(QT):
                                col = b * S + qt * 512
                                nc.tensor.matmul(
                                    pp[b * QT + qt][:],
                                    w_sb[:, o, :],
                                    xT_sb[:, o, col:col + 512],
                                    start=(o == 0),
                                    stop=(o == ET - 1),
                                )
                    for b in range(B):
                        for qt in range(QT):
                            p_t = pp[b * QT + qt]
                            q0 = qt * 512
                            if proj_i == 0:
                                nc.vector.tensor_scalar_add(
                                    qT_sb[:, b, q0:q0 + 512], p_t[:], b_sb[:]
                                )
                            elif proj_i == 1:
                                nc.vector.tensor_scalar_add(
                                    kT_sb[:, b, q0:q0 + 512], p_t[:], b_sb[:]
                                )
                            else:
                                nc.vector.tensor_scalar_add(
                                    vT_tmp[b][:, q0:q0 + 512], p_t[:], b_sb[:]
                                )

            with tc.tile_pool(name="psA2", bufs=1, space="PSUM") as psA2:
                # transpose V^T -> V (split heads into the padded layout);
                # colsum(V') = free-axis reduce of V^T on the DVE (the d axis
                # is already partition-major there) — no PE involvement.
                for b in range(B):
                    csum = ppool.tile([P, 1], fp32, name=f"csum_{b}")
                    nc.vector.tensor_reduce(
                        csum[:], vT_tmp[b][:], mybir.AxisListType.X, Alu.add
                    )
                    nc.vector.tensor_scalar_mul(
                        cc_sb[:, b * HPD:b * HPD + 1], csum[:], sc_sb[:, 1:2]
                    )
                    nc.vector.tensor_copy(
                        out=cc_sb[:, b * HPD + 1:b * HPD + 2],
                        in_=cc_sb[:, b * HPD:b * HPD + 1],
                    )
                    for kt in range(KT):
                        pt = psA2.tile(
                            [P, P], fp16, tag="tr", bufs=3, name=f"pt_{b}_{kt}"
                        )
                        nc.tensor.transpose(
                            pt[:], vT_tmp[b][:, kt * P:(kt + 1) * P], id_sb[:]
                        )
                        nc.vector.tensor_copy(
                            out=v_sb[:, b, kt, 0:Hd], in_=pt[:, 0:Hd]
                        )
                        nc.vector.tensor_copy(
                            out=v_sb[:, b, kt, 2 * Hd:3 * Hd], in_=pt[:, Hd:P]
                        )

            xpool.release()

            # ================= P2: evidence attention =================
            wpool = tc.alloc_tile_pool(name="work", bufs=1)
            # per (b, S-half) AllGather: fires as soon as the first two
            # q-tiles of both heads are done -> overlaps remaining compute
            ag_in = [
                [
                    dpool.tile([FPD, S // 2], fp16, name=f"ag_in_{b}_{sh}")
                    for sh in range(2)
                ]
                for b in range(B)
            ]
            ag_out = [
                [
                    dpool.tile(
                        [NCORES * FPD, S // 2], fp16, addr_space="Shared",
                        name=f"ag_out_{b}_{sh}",
                    )
                    for sh in range(2)
                ]
                for b in range(B)
            ]

            with tc.tile_pool(name="psB", bufs=1, space="PSUM") as psB:
                def _emit_p3(b):
                    # st-outer: one [128,512] accumulator at a time from the
                    # shared "ev" tag, so these matmuls interleave into the
                    # other batch's P2 stream as soon as the AllGather lands.
                    for st in range(QT):
                        po = psB.tile(
                            [P, 512], fp32, tag="ev", bufs=4,
                            name=f"po_{b}_{st}",
                        )
                        sh = st // 2
                        for fc in range(ET):
                            agc = wpool.tile(
                                [P, 512], fp16, tag="agc", bufs=4,
                                name=f"agc_{b}_{st}_{fc}",
                            )
                            nc.sync.dma_start(
                                agc[:],
                                ag_out[b][sh][fc * P:(fc + 1) * P,
                                              (st % 2) * 512:
                                              (st % 2) * 512 + 512],
                            )
                            nc.tensor.matmul(
                                po[:],
                                wo_sb[:, fc, :],
                                agc[:],
                                start=(fc == 0),
                                stop=(fc == ET - 1),
                            )
                        ot = wpool.tile(
                            [P, 512], fp32, tag="ot", bufs=3,
                            name=f"ot_{b}_{st}",
                        )
                        nc.vector.tensor_scalar_add(ot[:], po[:], bo_sb[:])
                        nc.sync.dma_start(
                            outT[:, b * S + st * 512:b * S + (st + 1) * 512],
                            ot[:],
                        )

                for b in range(B):
                    for qt in range(QT):
                        q0 = qt * 512
                        # one [128,1024] scores tile holds BOTH heads for
                        # this q-tile; the two 64-contraction matmuls run
                        # concurrently on disjoint PE row groups.
                        ev = [
                            psB.tile(
                                [P, 512], fp32, tag="ev", bufs=4,
                                name=f"ev_{b}_{qt}_{h}",
                            )
                            for h in range(HPD)
                        ]
                        for kt in range(KT):
                            ss = psB.tile(
                                [P, 1024], fp32, tag="sc", bufs=2,
                                name=f"ss_{b}_{qt}_{kt}",
                            )
                            nc.tensor.matmul(
                                ss[:, 0:512],
                                kT_sb[0:Hd, b, kt * P:(kt + 1) * P],
                                qT_sb[0:Hd, b, q0:q0 + 512],
                                start=True,
                                stop=True,
                                tile_position=(0, 0),
                            )
                            nc.tensor.matmul(
                                ss[:, 512:1024],
                                kT_sb[Hd:P, b, kt * P:(kt + 1) * P],
                                qT_sb[Hd:P, b, q0:q0 + 512],
                                start=True,
                                stop=True,
                                tile_position=(Hd, 0),
                            )
                            et = wpool.tile(
                                [P, 1024], fp16, tag="et", bufs=4,
                                name=f"et_{b}_{qt}_{kt}",
                            )
                            nc.scalar.activation(
                                et[:], ss[:], ActF.Exp, scale=0.125
                            )
                            for h in range(HPD):
                                nc.tensor.matmul(
                                    ev[h][:],
                                    v_sb[:, b, kt, h * Hd:h * Hd + P],
                                    et[:, h * 512:(h + 1) * 512],
                                    start=(kt == 0),
                                    stop=(kt == KT - 1),
                                )
                        # ---- epilogue for this (b, qt), both heads ----
                        for h in range(HPD):
                            col = b * HPD + h
                            e_lo = h * Hd          # EV rows base
                            d_lo = Hd - h * Hd     # rowsum rows base
                            den = wpool.tile(
                                [Hd, 512], fp32, tag="den", bufs=2,
                                name=f"den_{b}_{qt}_{h}",
                            )
                            tmp = wpool.tile(
                                [Hd, 512], fp32, tag="tmp", bufs=2,
                                name=f"tmp_{b}_{qt}_{h}",
                            )
                            nc.vector.tensor_scalar(
                                den[:],
                                ev[h][d_lo:d_lo + Hd, :],
                                sc_sb[d_lo:d_lo + Hd, 0:1],
                                sc_sb[d_lo:d_lo + Hd, 2:3],
                                Alu.mult,
                                Alu.add,
                            )
                            nc.vector.tensor_scalar(
                                tmp[:],
                                ev[h][e_lo:e_lo + Hd, :],
                                sc_sb[e_lo:e_lo + Hd, 0:1],
                                cc_sb[e_lo:e_lo + Hd, col:col + 1],
                                Alu.mult,
                                Alu.add,
                            )
                            rb = wpool.tile(
                                [Hd, 512], fp32, tag="rb", bufs=2,
                                name=f"rb_{b}_{qt}_{h}",
                            )
                            rs = wpool.tile(
                                [Hd, 512], fp32, tag="rs", bufs=2,
                                name=f"rs_{b}_{qt}_{h}",
                            )
                            nc.vector.reciprocal_approx_accurate(
                                rb[:], den[:], rs[:]
                            )
                            nc.vector.tensor_tensor(
                                at_sb[b][h][:, q0:q0 + 512],
                                tmp[:],
                                rb[:],
                                Alu.mult,
                            )
                            nc.vector.tensor_scalar_mul(
                                unc_sb[col][:, q0:q0 + 512],
                                rb[0:1, :],
                                float(S),
                            )
                        if qt % 2 == 1:
                            sh = qt // 2
                            s0 = sh * (S // 2)
                            for h in range(HPD):
                                nc.sync.dma_start(
                                    ag_in[b][sh][h * Hd:(h + 1) * Hd, :],
                                    at_sb[b][h][:, s0:s0 + S // 2],
                                )
                            nc.gpsimd.collective_compute(
                                "AllGather",
                                mybir.AluOpType.bypass,
                                replica_groups=[list(range(NCORES))],
                                ins=[ag_in[b][sh].opt()],
                                outs=[ag_out[b][sh].opt()],
                            )

                for b in range(B):
                    _emit_p3(b)
            for i in range(B * HPD):
                nc.sync.dma_start(unc[i:i + 1, :], unc_sb[i][:])
            wpool.release()

    nc.finalize()
    return nc


def _get_nc():
    if "nc" not in _CACHE:
        _CACHE["nc"] = _build_nc()
    return _CACHE["nc"]


def kernel(
    x, q_w, q_b, k_w, k_b, v_w, v_b, out_w, out_b,
    evidence_scale, evidence_bias,
):
    from concourse.bass_utils import run_bass_kernel_spmd

    x = np.asarray(x, dtype=np.float32)
    q_w = np.asarray(q_w, dtype=np.float32)
    k_w = np.asarray(k_w, dtype=np.float32)
    v_w = np.asarray(v_w, dtype=np.float32)
    out_w = np.asarray(out_w, dtype=np.float32)
    q_b = np.asarray(q_b, dtype=np.float32)
    k_b = np.asarray(k_b, dtype=np.float32)
    v_b = np.asarray(v_b, dtype=np.float32)
    out_b = np.asarray(out_b, dtype=np.float32)
    scale = float(np.asarray(evidence_scale).reshape(-1)[0])
    ebias = float(np.asarray(evidence_bias).reshape(-1)[0])

    xTh = np.ascontiguousarray(
        x.reshape(B * S, D).T
    ).astype(np.float16)                                   # [D, B*S]
    ident = np.eye(P, dtype=np.float16)
    sc_host = np.empty((P, 3), dtype=np.float32)
    sc_host[:, 0] = scale
    sc_host[:, 1] = 1.0 + ebias
    sc_host[:, 2] = float(S) * (1.0 + ebias)

    def _wlayout(a):
        # a: [FPD, D] weight shard; device wants w_sb[p, o, m] = a[m, o*P+p]
        return np.ascontiguousarray(
            a.reshape(FPD, ET, P).transpose(2, 1, 0).reshape(P, ET * FPD)
        ).astype(np.float16)

    in_maps = []
    for d in range(NCORES):
        f0 = d * FPD
        sl = slice(f0, f0 + FPD)
        in_maps.append({
            "xT": xTh,
            "wq": _wlayout(q_w[sl, :]),
            "wk": _wlayout(k_w[sl, :]),
            "wv": _wlayout(v_w[sl, :]),
            "wo": _wlayout(out_w[sl, :]),
            "bq": np.ascontiguousarray(q_b[sl].reshape(FPD, 1)),
            "bk": np.ascontiguousarray(k_b[sl].reshape(FPD, 1)),
            "bv": np.ascontiguousarray(v_b[sl].reshape(FPD, 1)),
            "bo": np.ascontiguousarray(out_b[sl].reshape(FPD, 1)),
            "sc": sc_host,
            "ident": ident,
        })

    nc = _get_nc()
    trace = bool(int(os.environ.get("EVQ_TRACE", "0")))
    res = run_bass_kernel_spmd(
        nc, in_maps, core_ids=list(range(NCORES)), trace=trace,
    )
    if trace and res.exec_time_ns is not None:
        print(f"HW exec time: {res.exec_time_ns} ns")
        if res.instructions_and_trace is not None:
            print(f"trace: {res.instructions_and_trace[1]}")
        _CACHE["last_result"] = res

    out = np.empty((B, S, D), dtype=np.float32)
    uncertainty = np.empty((B, H, S), dtype=np.float32)
    for d in range(NCORES):
        r = res.results[d]
        oT = r["outT"].reshape(FPD, B, S)          # [128, B, S]
        for b in range(B):
            out[b, :, d * FPD:(d + 1) * FPD] = oT[:, b, :].T
        u = r["unc"]                               # [B*HPD, S]
        for b in range(B):
            for h in range(HPD):
                uncertainty[b, d * HPD + h, :] = u[b * HPD + h, :]
    return out, uncertainty


# revision 25
# speedup vs baseline: 1.0925x; 1.0594x over previous
"""EvidenceQualityLayer Trainium2 kernel — 8-core head+batch parallel.

Sharding: each of the 8 cores owns 2 heads (128 features) of the Q/K/V
projections (column-parallel) and computes its heads' full SxS evidence
tensor for both batch elements in a flash-attention-style streaming loop.
The attention output shards (feature-major, [128, S] per batch) are
AllGather'ed across cores, after which each core computes a 128-column
slice of the output projection (column-parallel out_proj).  No all-reduce
is needed anywhere.

Math identity used (no softmax-max trick needed, raw exp):
  evidence = exp(s)*scale + (1+bias)
  denom    = scale*rowsum(exp(s)) + S*(1+bias)
  attn     = (scale*(E @ V') + (1+bias)*colsum(V')) / denom
  unc      = S / denom
where V' = V + v_b (probs rows sum to 1 exactly, so folding v_b into V
is exact).  rowsum(E) comes for free from a ones-column appended to V.

All matmuls are fp16 operands with fp32 PSUM accumulation (~1e-3 rel
err, 4x faster than fp32 on the PE).  Both P2 matmul shapes are padded
to use the full 128x128 PE array (zero rows in K^T, zero cols in the V
stationary): half-array matmuls never trip the PE clock-gate's activity
monitor and the whole phase runs at 1.2 GHz instead of 2.4.
"""

import os

import numpy as np

B = 2
S = 2048
D = 1024
H = 16
Hd = 64
NCORES = 8
P = 128
HPD = H // NCORES      # heads per device (2)
FPD = HPD * Hd         # features per device (128)
QT = 4                 # q tiles of 512 per sequence
KT = S // P            # k tiles of 128 (16)
ET = D // P            # embed contraction chunks (8)
VW = 192               # V stationary layout: V0(64) | ones(64) | V1(64)

_CACHE: dict = {}


def _build_nc():
    import concourse.bacc as bacc
    import concourse.mybir as mybir
    import concourse.tile as tile

    fp16 = mybir.dt.float16
    fp32 = mybir.dt.float32
    Alu = mybir.AluOpType
    ActF = mybir.ActivationFunctionType

    nc = bacc.Bacc(num_devices=NCORES, name="evq")

    # ---- I/O ----
    xT = nc.dram_tensor("xT", [D, B * S], fp16, kind="ExternalInput")
    wq = nc.dram_tensor("wq", [P, ET * FPD], fp16, kind="ExternalInput")
    wk = nc.dram_tensor("wk", [P, ET * FPD], fp16, kind="ExternalInput")
    wv = nc.dram_tensor("wv", [P, ET * FPD], fp16, kind="ExternalInput")
    wo = nc.dram_tensor("wo", [P, ET * FPD], fp16, kind="ExternalInput")
    bq = nc.dram_tensor("bq", [FPD, 1], fp32, kind="ExternalInput")
    bk = nc.dram_tensor("bk", [FPD, 1], fp32, kind="ExternalInput")
    bv = nc.dram_tensor("bv", [FPD, 1], fp32, kind="ExternalInput")
    bo = nc.dram_tensor("bo", [FPD, 1], fp32, kind="ExternalInput")
    # sc columns: 0 = evidence_scale, 1 = (1+evidence_bias), 2 = S*(1+bias)
    sc = nc.dram_tensor("sc", [P, 3], fp32, kind="ExternalInput")
    ident = nc.dram_tensor("ident", [P, P], fp16, kind="ExternalInput")

    outT = nc.dram_tensor("outT", [FPD, B * S], fp32, kind="ExternalOutput")
    unc = nc.dram_tensor("unc", [B * HPD, S], fp32, kind="ExternalOutput")

    with tile.TileContext(nc) as tc:
        with (
            tc.tile_pool(name="consts", bufs=1) as cpool,
            tc.tile_pool(name="proj", bufs=1) as ppool,
            tc.tile_pool(name="dram", bufs=1, space="DRAM") as dpool,
        ):
            # ---- constants into SBUF ----
            wq_sb = cpool.tile([P, ET, FPD], fp16, name="wq_sb")
            wk_sb = cpool.tile([P, ET, FPD], fp16, name="wk_sb")
            wv_sb = cpool.tile([P, ET, FPD], fp16, name="wv_sb")
            wo_sb = cpool.tile([P, ET, FPD], fp16, name="wo_sb")
            for w_sb, w_dr in ((wq_sb, wq), (wk_sb, wk), (wv_sb, wv), (wo_sb, wo)):
                nc.gpsimd.dma_start(
                    w_sb[:], w_dr[:].rearrange("p (o m) -> p o m", o=ET)
                )
            bq_sb = cpool.tile([FPD, 1], fp32, name="bq_sb")
            bk_sb = cpool.tile([FPD, 1], fp32, name="bk_sb")
            bv_sb = cpool.tile([FPD, 1], fp32, name="bv_sb")
            bo_sb = cpool.tile([FPD, 1], fp32, name="bo_sb")
            for b_sb, b_dr in ((bq_sb, bq), (bk_sb, bk), (bv_sb, bv), (bo_sb, bo)):
                nc.gpsimd.dma_start(b_sb[:], b_dr[:])
            sc_sb = cpool.tile([P, 3], fp32, name="sc_sb")
            nc.gpsimd.dma_start(sc_sb[:], sc[:])
            id_sb = cpool.tile([P, P], fp16, name="id_sb")
            nc.gpsimd.dma_start(id_sb[:], ident[:])
            ones_sb = cpool.tile([P, 1], fp16, name="ones_sb")
            nc.vector.memset(ones_sb[:], 1.0)
            srow_sb = cpool.tile([1, S], fp32, name="srow_sb")
            nc.vector.memset(srow_sb[:], float(S))

            # ---- persistent per-phase tiles ----
            qT_sb = ppool.tile([P, B, S], fp16, name="qT_sb")
            # K^T packed like Q^T: head h on partition rows [64h:64h+64]
            kT_sb = ppool.tile([P, B, S], fp16, name="kT_sb")
            # V stationary per (b, kt): [V0(64) | ones(64) | V1(64)].
            # Head slice h -> cols [64h : 64h+128] = [V_h | ones] (h0) or
            # [ones | V_h] (h1), so the EV matmul's 128-row PSUM output
            # carries EV on one 64-row half and 64 replicated rowsum(E)
            # rows on the other half -> full PE array AND no partition
            # broadcast needed for the denominator.
            v_sb = ppool.tile([P, B, KT, VW], fp16, name="v_sb")
            for b in range(B):
                for kt in range(KT):
                    nc.vector.memset(v_sb[:, b, kt, Hd:2 * Hd], 1.0)
            # (1+bias)*colsum(V') per (b, head), aligned to the head's
            # EV rows (h0 -> partitions 0:64, h1 -> partitions 64:128)
            cc_sb = ppool.tile([P, B * HPD], fp32, name="cc_sb")
            # attention output shards, feature-major [64, S] per (b, h)
            at_sb = [
                [
                    ppool.tile([Hd, S], fp16, name=f"at_{b}_{h}")
                    for h in range(HPD)
                ]
                for b in range(B)
            ]
            unc_sb = [
                ppool.tile([1, S], fp32, name=f"unc_sb_{i}")
                for i in range(B * HPD)
            ]

            # ================= P1: projections =================
            # o-outer loop: one stationary weight chunk feeds all 8
            # (b, qt) accumulators -> dense full-array MM stream.
            # xpool (x^T + V^T temps) is scoped to P1 so its 72 KB/partition
            # is reused by the P2/P3 working pool.
            xpool = tc.alloc_tile_pool(name="xp", bufs=1)
            xT_sb = xpool.tile([P, ET, B * S], fp16, name="xT_sb")
            for o in range(ET):
                nc.sync.dma_start(xT_sb[:, o, :], xT[o * P:(o + 1) * P, :])

            with tc.tile_pool(name="psA1", bufs=1, space="PSUM") as psA1:
                vT_tmp = [
                    xpool.tile([P, S], fp16, name=f"vT_{b}") for b in range(B)
                ]
                for proj_i, (w_sb, b_sb) in enumerate(
                    ((wq_sb, bq_sb), (wk_sb, bk_sb), (wv_sb, bv_sb))
                ):
                    pp = [
                        psA1.tile(
                            [P, 512], fp32, tag="proj", bufs=8,
                            name=f"pp_{proj_i}_{i}",
                        )
                        for i in range(B * QT)
                    ]
                    for o in range(ET):
                        for b in range(B):
                            for qt in range(QT):
                                col = b * S + qt * 512
                                nc.tensor.matmul(
                                    pp[b * QT + qt][:],
                                    w_sb[:, o, :],
                                    xT_sb[:, o, col:col + 512],
                                    start=(o == 0),
                                    stop=(o == ET - 1),
                                )
                    for b in range(B):
                        for qt in range(QT):
                            p_t = pp[b * QT + qt]
                            q0 = qt * 512
                            if proj_i == 0:
                                nc.vector.tensor_scalar_add(
                                    qT_sb[:, b, q0:q0 + 512], p_t[:], b_sb[:]
                                )
                            elif proj_i == 1:
                                nc.vector.tensor_scalar_add(
                                    kT_sb[:, b, q0:q0 + 512], p_t[:], b_sb[:]
                                )
                            else:
                                nc.vector.tensor_scalar_add(
                                    vT_tmp[b][:, q0:q0 + 512], p_t[:], b_sb[:]
                                )

            with tc.tile_pool(name="psA2", bufs=1, space="PSUM") as psA2:
                # transpose V^T -> V (split heads into the padded layout);
                # colsum(V') = free-axis reduce of V^T on the DVE (the d axis
                # is already partition-major there) — no PE involvement.
                for b in range(B):
                    csum = ppool.tile([P, 1], fp32, name=f"csum_{b}")
                    nc.vector.tensor_reduce(
                        csum[:], vT_tmp[b][:], mybir.AxisListType.X, Alu.add
                    )
                    nc.vector.tensor_scalar_mul(
                        cc_sb[:, b * HPD:b * HPD + 1], csum[:], sc_sb[:, 1:2]
                    )
                    nc.vector.tensor_copy(
                        out=cc_sb[:, b * HPD + 1:b * HPD + 2],
                        in_=cc_sb[:, b * HPD:b * HPD + 1],
                    )
                    for kt in range(KT):
                        pt = psA2.tile(
                            [P, P], fp16, tag="tr", bufs=3, name=f"pt_{b}_{kt}"
                        )
                        nc.tensor.transpose(
                            pt[:], vT_tmp[b][:, kt * P:(kt + 1) * P], id_sb[:]
                        )
                        nc.vector.tensor_copy(
                            out=v_sb[:, b, kt, 0:Hd], in_=pt[:, 0:Hd]
                        )
                        nc.vector.tensor_copy(
                            out=v_sb[:, b, kt, 2 * Hd:3 * Hd], in_=pt[:, Hd:P]
                        )

            xpool.release()

            # ================= P2: evidence attention =================
            wpool = tc.alloc_tile_pool(name="work", bufs=1)
            # per (b, S-half) AllGather: fires as soon as the first two
            # q-tiles of both heads are done -> overlaps remaining compute
            ag_in = [
                [
                    dpool.tile([FPD, 512], fp16, name=f"ag_in_{b}_{sh}")
                    for sh in range(QT)
                ]
                for b in range(B)
            ]
            ag_out = [
                [
                    dpool.tile(
                        [NCORES * FPD, 512], fp16, addr_space="Shared",
                        name=f"ag_out_{b}_{sh}",
                    )
                    for sh in range(QT)
                ]
                for b in range(B)
            ]

            with tc.tile_pool(name="psB", bufs=1, space="PSUM") as psB:
                def _emit_p3(b):
                    # st-outer: one [128,512] accumulator at a time from the
                    # shared "ev" tag, so these matmuls interleave into the
                    # other batch's P2 stream as soon as the AllGather lands.
                    for st in range(QT):
                        po = psB.tile(
                            [P, 512], fp32, tag="ev", bufs=4,
                            name=f"po_{b}_{st}",
                        )
                        for fc in range(ET):
                            agc = wpool.tile(
                                [P, 512], fp16, tag="agc", bufs=4,
                                name=f"agc_{b}_{st}_{fc}",
                            )
                            nc.sync.dma_start(
                                agc[:],
                                ag_out[b][st][fc * P:(fc + 1) * P, :],
                            )
                            nc.tensor.matmul(
                                po[:],
                                wo_sb[:, fc, :],
                                agc[:],
                                start=(fc == 0),
                                stop=(fc == ET - 1),
                            )
                        ot = wpool.tile(
                            [P, 512], fp32, tag="ot", bufs=3,
                            name=f"ot_{b}_{st}",
                        )
                        nc.vector.tensor_scalar_add(ot[:], po[:], bo_sb[:])
                        nc.sync.dma_start(
                            outT[:, b * S + st * 512:b * S + (st + 1) * 512],
                            ot[:],
                        )

                for b in range(B):
                    for qt in range(QT):
                        q0 = qt * 512
                        # one [128,1024] scores tile holds BOTH heads for
                        # this q-tile; the two 64-contraction matmuls run
                        # concurrently on disjoint PE row groups.
                        ev = [
                            psB.tile(
                                [P, 512], fp32, tag="ev", bufs=4,
                                name=f"ev_{b}_{qt}_{h}",
                            )
                            for h in range(HPD)
                        ]
                        for kt in range(KT):
                            ss = psB.tile(
                                [P, 1024], fp32, tag="sc", bufs=2,
                                name=f"ss_{b}_{qt}_{kt}",
                            )
                            nc.tensor.matmul(
                                ss[:, 0:512],
                                kT_sb[0:Hd, b, kt * P:(kt + 1) * P],
                                qT_sb[0:Hd, b, q0:q0 + 512],
                                start=True,
                                stop=True,
                                tile_position=(0, 0),
                            )
                            nc.tensor.matmul(
                                ss[:, 512:1024],
                                kT_sb[Hd:P, b, kt * P:(kt + 1) * P],
                                qT_sb[Hd:P, b, q0:q0 + 512],
                                start=True,
                                stop=True,
                                tile_position=(Hd, 0),
                            )
                            et = wpool.tile(
                                [P, 1024], fp16, tag="et", bufs=4,
                                name=f"et_{b}_{qt}_{kt}",
                            )
                            nc.scalar.activation(
                                et[:], ss[:], ActF.Exp, scale=0.125
                            )
                            for h in range(HPD):
                                nc.tensor.matmul(
                                    ev[h][:],
                                    v_sb[:, b, kt, h * Hd:h * Hd + P],
                                    et[:, h * 512:(h + 1) * 512],
                                    start=(kt == 0),
                                    stop=(kt == KT - 1),
                                )
                        # ---- epilogue for this (b, qt), both heads ----
                        for h in range(HPD):
                            col = b * HPD + h
                            e_lo = h * Hd          # EV rows base
                            d_lo = Hd - h * Hd     # rowsum rows base
                            den = wpool.tile(
                                [Hd, 512], fp32, tag="den", bufs=2,
                                name=f"den_{b}_{qt}_{h}",
                            )
                            tmp = wpool.tile(
                                [Hd, 512], fp32, tag="tmp", bufs=2,
                                name=f"tmp_{b}_{qt}_{h}",
                            )
                            nc.vector.tensor_scalar(
                                den[:],
                                ev[h][d_lo:d_lo + Hd, :],
                                sc_sb[d_lo:d_lo + Hd, 0:1],
                                sc_sb[d_lo:d_lo + Hd, 2:3],
                                Alu.mult,
                                Alu.add,
                            )
                            nc.vector.tensor_scalar(
                                tmp[:],
                                ev[h][e_lo:e_lo + Hd, :],
                                sc_sb[e_lo:e_lo + Hd, 0:1],
                                cc_sb[e_lo:e_lo + Hd, col:col + 1],
                                Alu.mult,
                                Alu.add,
                            )
                            rb = wpool.tile(
                                [Hd, 512], fp32, tag="rb", bufs=2,
                                name=f"rb_{b}_{qt}_{h}",
                            )
                            rs = wpool.tile(
                                [Hd, 512], fp32, tag="rs", bufs=2,
                                name=f"rs_{b}_{qt}_{h}",
                            )
                            nc.vector.reciprocal_approx_accurate(
                                rb[:], den[:], rs[:]
                            )
                            nc.vector.tensor_tensor(
                                at_sb[b][h][:, q0:q0 + 512],
                                tmp[:],
                                rb[:],
                                Alu.mult,
                            )
                            nc.vector.tensor_scalar_mul(
                                unc_sb[col][:, q0:q0 + 512],
                                rb[0:1, :],
                                float(S),
                            )
                            nc.sync.dma_start(
                                unc[col:col + 1, q0:q0 + 512],
                                unc_sb[col][:, q0:q0 + 512],
                            )
                        for h in range(HPD):
                            nc.sync.dma_start(
                                ag_in[b][qt][h * Hd:(h + 1) * Hd, :],
                                at_sb[b][h][:, q0:q0 + 512],
                            )
                        nc.gpsimd.collective_compute(
                            "AllGather",
                            mybir.AluOpType.bypass,
                            replica_groups=[list(range(NCORES))],
                            ins=[ag_in[b][qt].opt()],
                            outs=[ag_out[b][qt].opt()],
                        )

                for b in range(B):
                    _emit_p3(b)
            wpool.release()

    nc.finalize()
    return nc


def _get_nc():
    if "nc" not in _CACHE:
        _CACHE["nc"] = _build_nc()
    return _CACHE["nc"]


def kernel(
    x, q_w, q_b, k_w, k_b, v_w, v_b, out_w, out_b,
    evidence_scale, evidence_bias,
):
    from concourse.bass_utils import run_bass_kernel_spmd

    x = np.asarray(x, dtype=np.float32)
    q_w = np.asarray(q_w, dtype=np.float32)
    k_w = np.asarray(k_w, dtype=np.float32)
    v_w = np.asarray(v_w, dtype=np.float32)
    out_w = np.asarray(out_w, dtype=np.float32)
    q_b = np.asarray(q_b, dtype=np.float32)
    k_b = np.asarray(k_b, dtype=np.float32)
    v_b = np.asarray(v_b, dtype=np.float32)
    out_b = np.asarray(out_b, dtype=np.float32)
    scale = float(np.asarray(evidence_scale).reshape(-1)[0])
    ebias = float(np.asarray(evidence_bias).reshape(-1)[0])

    xTh = np.ascontiguousarray(
        x.reshape(B * S, D).T
    ).astype(np.float16)                                   # [D, B*S]
    ident = np.eye(P, dtype=np.float16)
    sc_host = np.empty((P, 3), dtype=np.float32)
    sc_host[:, 0] = scale
    sc_host[:, 1] = 1.0 + ebias
    sc_host[:, 2] = float(S) * (1.0 + ebias)

    def _wlayout(a):
        # a: [FPD, D] weight shard; device wants w_sb[p, o, m] = a[m, o*P+p]
        return np.ascontiguousarray(
            a.reshape(FPD, ET, P).transpose(2, 1, 0).reshape(P, ET * FPD)
        ).astype(np.float16)

    in_maps = []
    for d in range(NCORES):
        f0 = d * FPD
        sl = slice(f0, f0 + FPD)
        in_maps.append({
            "xT": xTh,
            "wq": _wlayout(q_w[sl, :]),
            "wk": _wlayout(k_w[sl, :]),
            "wv": _wlayout(v_w[sl, :]),
            "wo": _wlayout(out_w[sl, :]),
            "bq": np.ascontiguousarray(q_b[sl].reshape(FPD, 1)),
            "bk": np.ascontiguousarray(k_b[sl].reshape(FPD, 1)),
            "bv": np.ascontiguousarray(v_b[sl].reshape(FPD, 1)),
            "bo": np.ascontiguousarray(out_b[sl].reshape(FPD, 1)),
            "sc": sc_host,
            "ident": ident,
        })

    nc = _get_nc()
    trace = bool(int(os.environ.get("EVQ_TRACE", "0")))
    res = run_bass_kernel_spmd(
        nc, in_maps, core_ids=list(range(NCORES)), trace=trace,
    )
    if trace and res.exec_time_ns is not None:
        print(f"HW exec time: {res.exec_time_ns} ns")
        if res.instructions_and_trace is not None:
            print(f"trace: {res.instructions_and_trace[1]}")
        _CACHE["last_result"] = res

    out = np.empty((B, S, D), dtype=np.float32)
    uncertainty = np.empty((B, H, S), dtype=np.float32)
    for d in range(NCORES):
        r = res.results[d]
        oT = r["outT"].reshape(FPD, B, S)          # [128, B, S]
        for b in range(B):
            out[b, :, d * FPD:(d + 1) * FPD] = oT[:, b, :].T
        u = r["unc"]                               # [B*HPD, S]
        for b in range(B):
            for h in range(HPD):
                uncertainty[b, d * HPD + h, :] = u[b * HPD + h, :]
    return out, uncertainty
